# revision 8
# baseline (speedup 1.0000x reference)
"""Causal self-attention (B=4, T=2048, C=1024, H=16) on 8 TRN2 NeuronCores.

Sharding: batch x head-halves. Core i handles batch b=i//2 and heads
[8*(i%2), 8*(i%2)+8). Each core computes QKV projection for its slice,
causal attention for its 8 heads, and a partial output projection
(512 of 1024 contraction features). The host sums the two partials per
batch and transposes back.

All matmuls run in bf16 (fp32 accumulate): same 1 column/cycle PE rate
as fp32r but half the DMA/SBUF traffic and fast (FWL) weight loads.
Measured end-to-end rel err ~5e-3 vs the 2e-2 budget.

Attention per head-pair works on S^T tiles: S^T[k, q] so the AV matmul
(lhsT = V [k, d+1], rhs = P^T [k, q]) needs no transposes, with a ones
column appended to V so row 64 of the PSUM accumulator collects the
softmax denominators. exp runs on the scalar engine PSUM->SBUF with the
1/8 scale folded in. Causality: (k-tile, q-chunk) pairs above the
diagonal are skipped, diagonal tiles compute only valid columns and get
a triangular mask multiply on the 128-wide diagonal block.

Softmax normalization is DMA-free: the two denominator rows (PSUM row 64
of each head's O^T accumulator) are reciprocal'd by the vector engine
into single SBUF rows at partitions 64 and 32, then a pair of K=1
matmuls (lhsT = ones column) broadcasts each row across 64 PSUM
partitions; two vector multiplies produce the normalized, partition-
packed [128, CH] head-pair output the out-projection consumes. The
whole chain is ~2us of engine ops with no DMA round trip, so the
kernel tail (last head-pair -> final out-proj) is short.

Scheduling: the scalar-engine exp cadence (~1us per k-tile step) gates
attention, so phase2(j)'s kt loop is the pacing spine and matmul work
from other phases is injected INTO it as filler quanta (phase1 chunk
j+1 projection jobs, phase3 chunk j-1 out-proj jobs). This keeps the PE
busy during exp waits instead of idling ~40% of each attention span.
Phase1 chunk 0 runs kt-outer with 6 concurrent PSUM accumulators
(borrowing the attention-score pool) so compute starts as soon as the
first 384KB k-tile of weights lands instead of after the full 4MB.
"""

import sys

if "/opt/trn_rl_repo" not in sys.path:
    sys.path.insert(0, "/opt/trn_rl_repo")

import numpy as np
import ml_dtypes

import concourse.bass as bass
import concourse.mybir as mybir
import concourse.tile as tile
from concourse import bacc
from concourse.bass_utils import run_bass_kernel_spmd
from concourse.masks import make_upper_triangular

B, T, C, H = 4, 2048, 1024, 16
HD = C // H  # 64
NCORES = 8
HPC = H // 2  # heads per core = 8
F = HPC * HD  # 512 features per core
CH = 512  # t/q chunk width
NCH = T // CH  # 4
NKT = T // 128  # 16 k-tiles

f32 = mybir.dt.float32
f32r = mybir.dt.float32r
bf16 = mybir.dt.bfloat16


def build_nc():
    nc = bacc.Bacc("TRN2", target_bir_lowering=False, debug=False)
    xT = nc.dram_tensor("xT", [C, T], bf16, kind="ExternalInput").ap()
    wqk = nc.dram_tensor("wqk", [C, 2 * F], bf16, kind="ExternalInput").ap()
    wv = nc.dram_tensor("wv", [C, F], bf16, kind="ExternalInput").ap()
    wo = nc.dram_tensor("wo", [F, C], bf16, kind="ExternalInput").ap()
    yT = nc.dram_tensor("yT", [C, T], f32, kind="ExternalOutput").ap()

    with tile.TileContext(nc) as tc:
        with (
            tc.tile_pool(name="consts", bufs=1) as consts,
            tc.tile_pool(name="kv", bufs=1) as kv,
            tc.tile_pool(name="qtp", bufs=4) as qtp,
            tc.tile_pool(name="win", bufs=1) as win,
            tc.tile_pool(name="xin", bufs=2) as xin,
            tc.tile_pool(name="wout", bufs=1) as wout,
            tc.tile_pool(name="obuf", bufs=16) as obuf,
            tc.tile_pool(name="oun", bufs=2) as oun,
            tc.tile_pool(name="dnp", bufs=2) as dnp,
            tc.tile_pool(name="pexp", bufs=4) as pexp,
            tc.tile_pool(name="ysb", bufs=2) as ysb,
            tc.tile_pool(name="ps", bufs=2, space="PSUM") as ps,
            tc.tile_pool(name="pss", bufs=2, space="PSUM") as pss,
            tc.tile_pool(name="pso", bufs=2, space="PSUM") as pso,
        ):
            tri = consts.tile([128, 128], f32)
            make_upper_triangular(nc, tri[:], val=1.0, diag=True)

            kt_sb = kv.tile([128, 4, T], bf16)  # K^T; head 2p|2p+1 on parts 0-63|64-127
            v_sb = kv.tile([128, NKT, HPC, HD + 1], bf16)
            ones = consts.tile([128, NKT * HPC], f32)
            nc.vector.memset(ones[:], 1.0)
            nc.vector.tensor_copy(
                out=v_sb[:, :, :, HD : HD + 1],
                in_=ones[:].rearrange("p (a b c) -> p a b c", a=NKT, b=HPC),
            )
            # block-mask for the single denominator-broadcast matmul:
            # row 64 -> out partitions 0-63 (head 0), row 96 -> 64-127
            # (head 1), zeros elsewhere. Built via an f32 scratch because
            # walrus rejects memset directly into f32r tiles.
            blkf = consts.tile([128, 128], f32)
            nc.vector.memset(blkf[:], 0.0)
            nc.vector.tensor_copy(out=blkf[64:65, 0:64], in_=ones[64:65, 0:64])
            nc.vector.tensor_copy(out=blkf[96:97, 64:128], in_=ones[96:97, 64:128])
            blk = consts.tile([128, 128], f32r)
            nc.vector.tensor_copy(out=blk[:], in_=blkf[:])
            # persistent denominator-row tiles (rows 65-95 stay zero so the
            # K=33 broadcast matmul's dead rows contribute nothing)
            zf = consts.tile([128, CH], f32)
            nc.vector.memset(zf[:], 0.0)

            # spin the PE through the HAM activity window while the first
            # input DMAs land, so phase1 starts at 2.4GHz, not 1.2GHz.
            warm = consts.tile([128, 128], f32r)
            nc.vector.tensor_copy(out=warm[:], in_=ones[:, 0:128])
            wps = ps.tile([128, CH], f32, name="warmps", tag="mm")
            for i in range(24):
                nc.tensor.matmul(
                    wps[:, 0:128], warm[:], warm[:], start=True, stop=True
                )

            # startup DMAs, interleaved per k-tile so phase1 chunk 0's
            # kt-outer loop can start after the first ~384KB
            xt_first = xin.tile([128, 8, CH], bf16, name="xt0", tag="xt")
            wqk_sb = win.tile([128, 8, 2 * F], bf16)
            wv_sb = win.tile([128, 8, F], bf16)
            for kt in range(8):
                nc.sync.dma_start(
                    out=wqk_sb[:, kt, 0:F], in_=wqk[kt * 128 : (kt + 1) * 128, 0:F]
                )
                nc.sync.dma_start(
                    out=xt_first[:, kt, :],
                    in_=xT[kt * 128 : (kt + 1) * 128, 0:CH],
                )
                nc.sync.dma_start(
                    out=wv_sb[:, kt, :], in_=wv[kt * 128 : (kt + 1) * 128, :]
                )
            for kt in range(8):
                nc.sync.dma_start(
                    out=wqk_sb[:, kt, F : 2 * F],
                    in_=wqk[kt * 128 : (kt + 1) * 128, F : 2 * F],
                )
            wo_sb = wout.tile([128, 4, C], bf16)

            qt_tiles = {}
            osb_tiles = {}
            xt_tiles = {0: xt_first}

            def xt_dma(j):
                t0 = j * CH
                xt = xin.tile([128, 8, CH], bf16, name=f"xt{j}", tag="xt")
                nc.sync.dma_start(
                    out=xt[:],
                    in_=xT[:, t0 : t0 + CH].rearrange("(kt p) t -> p kt t", p=128),
                )
                xt_tiles[j] = xt

            def phase1_chunk0():
                """kt-outer over 6 concurrent accumulators (4 QK + 2 V),
                in two passes, so compute overlaps the startup DMA."""
                xt = xt_tiles[0]
                qt = qtp.tile([128, 4, CH], bf16, name="qt0", tag="qt")
                qt_tiles[0] = qt
                for half in range(2):
                    sA = pss.tile([128, 2, CH], f32, name=f"p10a{half}", tag="s")
                    sB = pss.tile([128, 2, CH], f32, name=f"p10b{half}", tag="s")
                    accs = [sA[:, 0, :], sA[:, 1, :], sB[:, 0, :], sB[:, 1, :]]
                    vaccs = [
                        ps.tile([128, CH], f32, name=f"p10v{half}_{s}", tag="mm")
                        for s in range(2)
                    ]
                    for kt in range(8):
                        for m in range(4):
                            mg = 4 * half + m
                            nc.tensor.matmul(
                                accs[m],
                                wqk_sb[:, kt, mg * 128 : (mg + 1) * 128],
                                xt[:, kt, :],
                                start=(kt == 0),
                                stop=(kt == 7),
                            )
                        for s in range(2):
                            sg = 2 * half + s
                            nc.tensor.matmul(
                                vaccs[s],
                                xt[:, kt, sg * 128 : (sg + 1) * 128],
                                wv_sb[:, kt, :],
                                start=(kt == 0),
                                stop=(kt == 7),
                            )
                    for m in range(4):
                        if half == 0:
                            nc.vector.tensor_copy(out=qt[:, m, :], in_=accs[m])
                        else:
                            nc.vector.tensor_copy(
                                out=kt_sb[:, m, 0:CH], in_=accs[m]
                            )
                    for s in range(2):
                        nc.vector.tensor_copy(
                            out=v_sb[:, 2 * half + s, :, 0:HD],
                            in_=vaccs[s].rearrange("p (h d) -> p h d", h=HPC),
                        )

            def phase1_gen(j, qk_ms=None, v_ss=None):
                """Generator of filler quanta: QK + V jobs, each 8
                matmuls + a PSUM->SBUF copy (~1.7us of PE work)."""
                t0 = j * CH
                xt = xt_tiles[j]
                if j not in qt_tiles:
                    qt_tiles[j] = qtp.tile(
                        [128, 4, CH], bf16, name=f"qt{j}", tag="qt"
                    )
                qt = qt_tiles[j]
                for m in (range(8) if qk_ms is None else qk_ms):
                    acc = ps.tile([128, CH], f32, name=f"qk_{j}_{m}", tag="mm")
                    for kt in range(8):
                        nc.tensor.matmul(
                            acc[:],
                            wqk_sb[:, kt, m * 128 : (m + 1) * 128],
                            xt[:, kt, :],
                            start=(kt == 0),
                            stop=(kt == 7),
                        )
                    if m < 4:
                        nc.vector.tensor_copy(out=qt[:, m, :], in_=acc[:])
                    else:
                        nc.vector.tensor_copy(
                            out=kt_sb[:, m % 4, t0 : t0 + CH], in_=acc[:]
                        )
                    yield
                for s in (range(CH // 128) if v_ss is None else v_ss):
                    accv = ps.tile([128, F], f32, name=f"v_{j}_{s}", tag="mm")
                    for kt in range(8):
                        nc.tensor.matmul(
                            accv[:],
                            xt[:, kt, s * 128 : (s + 1) * 128],
                            wv_sb[:, kt, :],
                            start=(kt == 0),
                            stop=(kt == 7),
                        )
                    nc.vector.tensor_copy(
                        out=v_sb[:, 4 * j + s, :, 0:HD],
                        in_=accv[:].rearrange("p (h d) -> p h d", h=HPC),
                    )
                    yield

            def phase3_gen(j):
                """Generator of filler quanta: 8 out-proj jobs, each 4
                matmuls + copy + store DMA (~0.85us of PE work)."""
                q0 = j * CH
                for ot in range(8):
                    acc = ps.tile([128, CH], f32, name=f"y_{j}_{ot}", tag="mm")
                    for ft in range(4):
                        nc.tensor.matmul(
                            acc[:],
                            wo_sb[:, ft, ot * 128 : (ot + 1) * 128],
                            osb_tiles[j][ft][:, :],
                            start=(ft == 0),
                            stop=(ft == 3),
                        )
                    y = ysb.tile([128, CH], f32, name=f"ysb_{j}_{ot}", tag="y")
                    nc.vector.tensor_copy(out=y[:], in_=acc[:])
                    nc.sync.dma_start(
                        out=yT[ot * 128 : (ot + 1) * 128, q0 : q0 + CH], in_=y[:]
                    )
                    yield

            def phase2(j, fillers=None, every=6, pace=None):
                q0 = j * CH
                nkt = 4 * j + 4
                osb_tiles[j] = []
                step = 0
                if pace is None:
                    pace = lambda s: 1 if s % every == 0 else 0
                for hp in range(4):
                    o_sb = obuf.tile(
                        [128, CH], bf16, name=f"osb{j}_{hp}", tag="osb"
                    )
                    osb_tiles[j].append(o_sb)
                    o_ps = [
                        pso.tile([HD + 1, CH], f32, name=f"o_{j}_{hp}_{hf}", tag="o")
                        for hf in range(2)
                    ]

                    def av(kt, p_t, lo):
                        for hf in range(2):
                            h = 2 * hp + hf
                            nc.tensor.matmul(
                                o_ps[hf][:, lo:CH],
                                v_sb[:, kt, h, :],
                                p_t[:, hf, lo:CH],
                                start=(kt == 0),
                                stop=(kt == nkt - 1),
                            )

                    prev = None
                    for kt in range(nkt):
                        k0 = kt * 128
                        lo = max(k0 - q0, 0)
                        # both heads' scores in one 2-bank PSUM tile so a
                        # single wide exp covers the pair; the two K=64
                        # matmuls use disjoint PE row groups.
                        s_t = pss.tile(
                            [128, 2, CH], f32, name=f"s_{j}_{hp}_{kt}", tag="s"
                        )
                        p_t = pexp.tile(
                            [128, 2, CH], bf16, name=f"p_{j}_{hp}_{kt}", tag="p"
                        )
                        for hf in range(2):
                            pb = hf * 64
                            nc.tensor.matmul(
                                s_t[:, hf, lo:CH],
                                kt_sb[pb : pb + 64, hp, k0 : k0 + 128],
                                qt_tiles[j][pb : pb + 64, hp, lo:CH],
                                start=True,
                                stop=True,
                            )
                        nc.scalar.activation(
                            out=p_t[:, :, lo:CH],
                            in_=s_t[:, :, lo:CH],
                            func=mybir.ActivationFunctionType.Exp,
                            scale=0.125,
                        )
                        if k0 >= q0:
                            for hf in range(2):
                                nc.vector.tensor_mul(
                                    out=p_t[:, hf, lo : lo + 128],
                                    in0=p_t[:, hf, lo : lo + 128],
                                    in1=tri[:],
                                )
                        # AV runs one k-tile behind so exp(kt) overlaps it
                        if prev is not None:
                            av(*prev)
                        prev = (kt, p_t, lo)
                        step += 1
                        if fillers is not None:
                            for _ in range(pace(step)):
                                next(fillers, None)
                    av(*prev)
                    # normalization, DMA-free: reciprocal the two PSUM
                    # denominator rows into partitions 64/32, broadcast
                    # across partitions with two K=1 matmuls, multiply.
                    dn = dnp.tile([128, CH], f32r, name=f"dn{j}_{hp}", tag="dn")
                    nc.vector.tensor_copy(out=dn[64:97, :], in_=zf[64:97, :])
                    # f32r is full fp32 storage; only the PE's later TF32
                    # read rounds (2.8e-4 rel on the recip values).
                    with nc.allow_low_precision(reason="f32r stores fp32 bits"):
                        nc.vector.reciprocal(
                            out=dn[64:65, :], in_=o_ps[0][64:65, :]
                        )
                        nc.vector.reciprocal(
                            out=dn[96:97, :], in_=o_ps[1][64:65, :]
                        )
                    o_un = oun.tile(
                        [128, CH], f32, name=f"oun_{j}_{hp}", tag="oun"
                    )
                    nc.vector.tensor_copy(out=o_un[0:64, :], in_=o_ps[0][0:64, :])
                    nc.vector.tensor_copy(out=o_un[64:128, :], in_=o_ps[1][0:64, :])
                    # one K=33 matmul broadcasts both recip rows across
                    # partitions: blk row 64 -> out 0-63, row 96 -> 64-127
                    # (matmul dst must start at partition 0 -- walrus
                    # rejects dst base 64, hence the combined form).
                    bc = ps.tile([128, CH], f32, name=f"bc{j}_{hp}", tag="mm")
                    nc.tensor.matmul(
                        bc[:, :], blk[64:97, :], dn[64:97, :],
                        start=True, stop=True,
                    )
                    nc.vector.tensor_mul(
                        out=o_sb[0:64, :], in0=o_un[0:64, :], in1=bc[0:64, :]
                    )
                    nc.vector.tensor_mul(
                        out=o_sb[64:128, :], in0=o_un[64:128, :], in1=bc[64:128, :]
                    )

            def drain(g):
                for _ in g:
                    pass

            # ---- emission schedule ----
            phase1_chunk0()
            xt_dma(1)
            nc.sync.dma_start(
                out=wo_sb[:], in_=wo.rearrange("(ft p) o -> p ft o", p=128)
            )
            drain(phase1_gen(1))
            xt_dma(2)
            f = phase1_gen(2)
            phase2(0, fillers=f, every=8)
            drain(f)
            xt_dma(3)
            f = phase1_gen(3, qk_ms=range(0, 4), v_ss=[])
            phase2(1, fillers=f, every=8)
            drain(f)
            f = _chain(phase3_gen(0), phase3_gen(1))
            phase2(2, fillers=f, every=3)
            drain(f)
            # K-tiles and V of chunk 3 are first consumed at kt=12 of each
            # head-pair, so they inject at the head of p2(3) (1/step) and
            # p3(2) jobs pace out the rest of the span.
            f = _chain(
                phase1_gen(3, qk_ms=range(4, 8), v_ss=range(4)),
                phase3_gen(2),
            )
            phase2(3, fillers=f, every=8, pace=lambda s: 1 if s <= 8 or s % 8 == 0 else 0)
            drain(f)
            drain(phase3_gen(3))

    nc.compile()
    return nc


def _chain(*gens):
    for g in gens:
        yield from g


def shard_inputs(x, W_qkv, W_out):
    """Build the 8 per-core input maps (bf16)."""

    def b16(a):
        return np.ascontiguousarray(a, dtype=np.float32).astype(ml_dtypes.bfloat16)

    xT = [b16(x[b].T) for b in range(B)]
    maps = []
    for core in range(NCORES):
        b, hf = core // 2, core % 2
        wq = W_qkv[:, hf * F : (hf + 1) * F]
        wk = W_qkv[:, C + hf * F : C + (hf + 1) * F]
        wvs = W_qkv[:, 2 * C + hf * F : 2 * C + (hf + 1) * F]
        maps.append(
            {
                "xT": xT[b],
                "wqk": b16(np.concatenate([wq, wk], axis=1)),
                "wv": b16(wvs),
                "wo": b16(W_out[hf * F : (hf + 1) * F, :]),
            }
        )
    return maps


_NC_CACHE = {}


def get_nc():
    if "nc" not in _NC_CACHE:
        _NC_CACHE["nc"] = build_nc()
    return _NC_CACHE["nc"]


def kernel(x, W_qkv, W_out, _run_kwargs=None):
    x = np.asarray(x, dtype=np.float32)
    W_qkv = np.asarray(W_qkv, dtype=np.float32)
    W_out = np.asarray(W_out, dtype=np.float32)
    nc = get_nc()
    maps = shard_inputs(x, W_qkv, W_out)
    res = run_bass_kernel_spmd(nc, maps, list(range(NCORES)), **(_run_kwargs or {}))
    out = np.empty((B, T, C), dtype=np.float32)
    for b in range(B):
        yT0 = res.results[2 * b]["yT"]
        yT1 = res.results[2 * b + 1]["yT"]
        out[b] = (yT0 + yT1).T
    if _run_kwargs is not None:
        _NC_CACHE["last_results"] = res
    return out
